# revision 1
# baseline (speedup 1.0000x reference)
"""Multi-head self-attention (ANE-style 1x1-conv attention) on 8 trn2 cores.

Sharding: zero-communication split over (batch, L-half). Core c handles
batch b = c//2 and query positions [half*1024, half*1024+1024) where
half = c%2. Each core computes k/v over the full L (keys/values are
needed for every query), so k/v projection work is duplicated 2x --
the price of avoiding any cross-core collective.

Per-core pipeline (fp16 operands everywhere, fp32 PSUM accumulation):
  1. vT = xT @ wvT -> v_spill (DRAM, fp16; DRAM roundtrip implements the
     transpose needed to get j on partitions for the AV matmul)
  2. q = wq @ x_half -> resident SBUF tiles (o-major)
  3. k = wk @ x -> k_spill (DRAM, fp16)
  4. per head-pair: sT_chunk = kz_h^T q_pair (j on partitions, i free),
     p = exp(sT/8) via ACT (the bottleneck engine), O += vaug^T p.
     kz/vaug are zero-padded to full 128-wide shapes: half-array matmuls
     keep the PE HAM throttled at K=4/8; the zero rows are numerically
     inert but make the array read as busy (measured 1.5x speedup).
     vaug also carries a ones column so O row 64 accumulates the softmax
     denominator. AV matmuls trail the scores by 2 iterations so the PE
     never stalls on the exp ACTs.
     Normalization runs off the PE: denom row -> DRAM -> (128,8) SBUF,
     reciprocal on 128 lanes, -> DRAM -> partition-broadcast load,
     one DVE multiply into the resident O tile.
  5. yT = O^T-proj: lhsT = resident O chunks, rhs = woT (+bias) -> yT.

All tile pools are kernel-scoped so the Tile scheduler can overlap the
phases by dataflow alone. DMA queues: sync = bulk loads, scalar (ACT
HWDGE) = attention-phase loads, gpsimd SWDGE = stores + init.

Host gathers: out[b, :, 0, half] = yT.T per core.
"""

import numpy as np

import concourse.bass as bass
import concourse.tile as tile
from concourse import bacc, mybir
from concourse.bass_utils import run_bass_kernel_spmd

B, D, L, H, Dh = 4, 1024, 2048, 16, 64
LH = L // 2  # per-core query range
NCORES = 8
FPR = mybir.dt.float32r
F32 = mybir.dt.float32
F16 = mybir.dt.float16
ACT_EXP = mybir.ActivationFunctionType.Exp
INV_SCALE = 1.0 / 8.0  # 1/sqrt(Dh)

NP = D // 128   # 8 partition-chunks of the model dim
NJC = L // 128  # 16 key chunks per head
NPAIR = H // 2  # 8 head pairs


def build_nc():
    nc = bacc.Bacc()
    x = nc.dram_tensor("x", [D, L], F16, kind="ExternalInput")
    xq = nc.dram_tensor("xq", [D, LH], F16, kind="ExternalInput")
    wqT = nc.dram_tensor("wqT", [D, D], F16, kind="ExternalInput")
    wkT = nc.dram_tensor("wkT", [D, D], F16, kind="ExternalInput")
    wvT = nc.dram_tensor("wvT", [D, D], F16, kind="ExternalInput")
    woT = nc.dram_tensor("woT", [D, D], F16, kind="ExternalInput")
    bo = nc.dram_tensor("bo", [1, D], F32, kind="ExternalInput")
    ones16 = nc.dram_tensor("ones16", [1, NJC], F16, kind="ExternalInput")
    zeros16 = nc.dram_tensor("zeros16", [1, L], F16, kind="ExternalInput")
    yT = nc.dram_tensor("yT", [LH, D], F32, kind="ExternalOutput")

    with tile.TileContext(nc) as tc:
        with (
            nc.allow_low_precision(reason="fp16 operands by design"),
            tc.tile_pool(name="dram", bufs=1, space="DRAM") as dram,
            tc.tile_pool(name="keep", bufs=1) as keep,
            tc.tile_pool(name="proj", bufs=1) as proj,
            tc.tile_pool(name="attn", bufs=1) as attn,
            tc.tile_pool(name="oproj", bufs=1) as oproj,
            tc.tile_pool(name="ps", bufs=2, space="PSUM") as ps,
        ):
            k_spill = dram.tile([D, L], F16)
            # v_spill holds vaug-ready blocks: per pair (128 j-part, NJC,
            # 256) where cols 0:64 / 128:192 are the two heads' values,
            # col 64/192 are ones (softmax denominator trick) and the rest
            # zeros (full-array padding). ones/zeros written once at init.
            v_spill = dram.tile([NPAIR, 128, NJC, 256], F16)

            zb = bass.AP(tensor=zeros16, offset=0, ap=[[0, 64], [1, L]])
            kz = [[None, None], [None, None]]
            for bi in range(2):
                for e in range(2):
                    t_ = attn.tile([128, L], F16, name="kz",
                                   tag=f"kz{bi}{e}", bufs=1)
                    nc.gpsimd.dma_start(
                        out=t_[64 * (1 - e):64 * (2 - e), :], in_=zb)
                    kz[bi][e] = t_

            vinit = attn.tile([128, NJC, 256], F16, name="vinit",
                              tag="vaug", bufs=3)
            nc.gpsimd.dma_start(
                out=vinit[:, :, 64:65],
                in_=bass.AP(tensor=ones16, offset=0,
                            ap=[[0, 128], [1, NJC], [1, 1]]))
            nc.gpsimd.dma_start(
                out=vinit[:, :, 65:128],
                in_=bass.AP(tensor=zeros16, offset=0,
                            ap=[[0, 128], [63, NJC], [1, 63]]))
            for t in range(NPAIR):
                for e in range(2):
                    nc.gpsimd.dma_start(
                        out=v_spill[t, :, :, 128 * e + 64:128 * e + 128],
                        in_=vinit[:, :, 64:128])

            # two psum tags, 2 slots x (128,1024) each = 8 banks total
            def ps_tile(tag):
                return ps.tile([128, 1024], F32, name="pst", tag=tag, bufs=2)

            # ---------------- projections ----------------
            xqk = []
            for t in range(NP):
                xt = proj.tile([128, LH], F16, name=f"xq{t}", tag=f"xq{t}")
                nc.sync.dma_start(out=xt, in_=xq[128 * t:128 * (t + 1), :])
                xqk.append(xt)
            # first q-projection weight block goes out first on the scalar
            # queue so the PE can start ~15us in; xk follows it
            wq0_t = []
            for kc in range(NP):
                wt = proj.tile([128, 128], F16, name="wq_t", tag="wq",
                               bufs=8)
                nc.scalar.dma_start(
                    out=wt, in_=wqT[128 * kc:128 * (kc + 1), 0:128])
                wq0_t.append(wt)
            xk = []
            for t in range(NP):
                xt = proj.tile([128, L], F16, name=f"xk{t}", tag=f"xk{t}")
                eng = nc.scalar if t % 2 == 0 else nc.sync
                eng.dma_start(out=xt, in_=x[128 * t:128 * (t + 1), :])
                xk.append(xt)

            # vT projection: out (l, o) -> v_spill (fp16)
            wv_t = []
            for kc in range(NP):
                wt = proj.tile([128, D], F16, name="wv_t", tag="wv", bufs=NP)
                nc.sync.dma_start(out=wt, in_=wvT[128 * kc:128 * (kc + 1), :])
                wv_t.append(wt)
            def emit_vproj_group(n, ml):
                    v_ps = ps_tile("ps_s")
                    for kc in range(NP):
                        nc.tensor.matmul(
                            v_ps[:, 0:512],
                            lhsT=xk[kc][:, 128 * ml:128 * (ml + 1)],
                            rhs=wv_t[kc][:, 512 * n:512 * (n + 1)],
                            start=(kc == 0), stop=(kc == NP - 1))
                    vsb = proj.tile([128, 512], F16, name="vsb", tag="vsb",
                                    bufs=2)
                    nc.vector.tensor_copy(out=vsb, in_=v_ps[:, 0:512])
                    # scatter (p, pairsub, ch) into the blocked layout,
                    # one DMA per head parity (4D APs unsupported)
                    vsb_r = vsb.rearrange("p (g c) -> p g c", g=4)
                    for e in range(2):
                        dst = bass.AP(
                            tensor=v_spill.tensor,
                            offset=v_spill.offset
                            + 4 * n * (128 * NJC * 256) + ml * 256 + 128 * e,
                            ap=[[NJC * 256, 128], [128 * NJC * 256, 4],
                                [1, 64]])
                        nc.gpsimd.dma_start(
                            out=dst, in_=vsb_r[:, :, 64 * e:64 * (e + 1)])

            def emit_vproj(n):
                for ml in range(L // 128):
                    emit_vproj_group(n, ml)

            # q projection (local L-half): out (o, i) -> resident SBUF
            q_res = []
            for mo in range(NP):
                qr = attn.tile([128, LH], F16, name=f"qres{mo}",
                               tag=f"qres{mo}")
                q_res.append(qr)
            def emit_qproj_group(mo, n):
                if mo == 0:
                    wq_t = wq0_t
                else:
                    wq_t = []
                    for kc in range(NP):
                        wt = proj.tile([128, 128], F16, name="wq_t",
                                       tag="wq", bufs=8)
                        nc.sync.dma_start(
                            out=wt, in_=wqT[128 * kc:128 * (kc + 1),
                                            128 * mo:128 * (mo + 1)])
                        wq_t.append(wt)
                q_ps = ps_tile("ps_s")
                for kc in range(NP):
                    nc.tensor.matmul(
                        q_ps[:, 0:512], lhsT=wq_t[kc],
                        rhs=xqk[kc][:, 512 * n:512 * (n + 1)],
                        start=(kc == 0), stop=(kc == NP - 1))
                nc.vector.tensor_copy(
                    out=q_res[mo][:, 512 * n:512 * (n + 1)],
                    in_=q_ps[:, 0:512])

            def emit_qproj(mo):
                for n in range(LH // 512):
                    emit_qproj_group(mo, n)

            # k projection: out (o, l) -> k_spill (fp16)
            def emit_kproj_group(mo, n):
                wk_t = []
                for kc in range(NP):
                    wt = proj.tile([128, 128], F16, name="wk_t", tag="wk",
                                   bufs=8)
                    nc.sync.dma_start(
                        out=wt, in_=wkT[128 * kc:128 * (kc + 1),
                                        128 * mo:128 * (mo + 1)])
                    wk_t.append(wt)
                k_ps = ps_tile("ps_s")
                for kc in range(NP):
                    nc.tensor.matmul(
                        k_ps[:, 0:512], lhsT=wk_t[kc],
                        rhs=xk[kc][:, 512 * n:512 * (n + 1)],
                        start=(kc == 0), stop=(kc == NP - 1))
                if mo <= 1:
                    # pairs 0/1 skip the DRAM roundtrip: copy straight
                    # into the padded kz tiles (zeros already in place)
                    nc.vector.tensor_copy(
                        out=kz[mo][0][0:64, 512 * n:512 * (n + 1)],
                        in_=k_ps[0:64, 0:512])
                    nc.vector.tensor_copy(
                        out=kz[mo][1][64:128, 512 * n:512 * (n + 1)],
                        in_=k_ps[64:128, 0:512])
                else:
                    ksb = proj.tile([128, 512], F16, name="ksb", tag="ksb",
                                    bufs=3)
                    nc.vector.tensor_copy(out=ksb, in_=k_ps[:, 0:512])
                    nc.gpsimd.dma_start(
                        out=k_spill[128 * mo:128 * (mo + 1),
                                    512 * n:512 * (n + 1)],
                        in_=ksb)

            def emit_kproj(mo):
                for n in range(L // 512):
                    emit_kproj_group(mo, n)

            # ---------------- attention ----------------
            o_res = []
            for t in range(NPAIR):
                orr = attn.tile([128, LH], F16, name=f"ores{t}",
                                tag=f"ores{t}")
                o_res.append(orr)

            def emit_pair(t, fillers=()):
                he, ho = 2 * t, 2 * t + 1
                bi = t % 2
                fillers = list(fillers)
                if t >= 2:
                    for e in range(2):
                        nc.scalar.dma_start(
                            out=kz[bi][e][64 * e:64 * (e + 1), :],
                            in_=k_spill[128 * t + 64 * e:
                                        128 * t + 64 * (e + 1), :])
                vaug = attn.tile([128, NJC, 256], F16, name="vaug",
                                 tag="vaug", bufs=3)
                nc.scalar.dma_start(out=vaug, in_=v_spill[t])
                q_pair = q_res[t]

                o_ps = [ps_tile("ps_o"), ps_tile("ps_o")]

                # software-pipelined: AV matmuls run 2 iterations behind
                # the scores so the PE never waits on the exp ACTs
                pts = {}

                def emit_scores(jc):
                    s_ps = [ps_tile("ps_s"), ps_tile("ps_s")]
                    for e in range(2):
                        lhsT = kz[bi][e][:, 128 * jc:128 * (jc + 1)]
                        for n in range(2):
                            nc.tensor.matmul(
                                s_ps[e][:, 512 * n:512 * (n + 1)],
                                lhsT=lhsT,
                                rhs=q_pair[:, 512 * n:512 * (n + 1)],
                                start=True, stop=True)
                    for e in range(2):
                        pt = attn.tile([128, LH], F16, name="pt", tag="pt",
                                       bufs=8)
                        nc.scalar.activation(pt, s_ps[e], ACT_EXP,
                                             scale=INV_SCALE)
                        pts[(jc, e)] = pt

                def emit_av(jc):
                    for e in range(2):
                        pt = pts.pop((jc, e))
                        for n in range(2):
                            nc.tensor.matmul(
                                o_ps[e][:, 512 * n:512 * (n + 1)],
                                lhsT=vaug[:, jc, 128 * e:128 * (e + 1)],
                                rhs=pt[:, 512 * n:512 * (n + 1)],
                                start=(jc == 0), stop=(jc == NJC - 1),
                                skip_group_check=True)

                for jc in range(NJC):
                    emit_scores(jc)
                    if jc >= 2:
                        emit_av(jc - 2)
                    if jc >= 2 and fillers:
                        fillers.pop(0)()
                emit_av(NJC - 2)
                emit_av(NJC - 1)
                while fillers:
                    fillers.pop(0)()

                # normalize off the PE: denom row -> DRAM -> 128-lane
                # reciprocal -> DRAM -> partition-broadcast -> DVE multiply
                for e in range(2):
                    osb_raw = attn.tile([65, LH], F16, name="osb_raw",
                                        tag=f"osb_raw{e}", bufs=1)
                    nc.vector.tensor_copy(out=osb_raw, in_=o_ps[e][0:65, :])
                    dnd = dram.tile([1, LH], F32, name="dnd",
                                    tag=f"dnd{t}_{e}")
                    nc.gpsimd.dma_start(out=dnd, in_=osb_raw[64:65, :])
                    dsc = attn.tile([128, LH // 128], F32, name="dsc",
                                    tag=f"dsc{e}", bufs=2)
                    nc.scalar.dma_start(
                        out=dsc,
                        in_=dnd.rearrange("o (a b) -> a o b", a=128))
                    rsc = attn.tile([128, LH // 128], F16, name="rsc",
                                    tag=f"rsc{e}", bufs=2)
                    nc.vector.reciprocal(out=rsc, in_=dsc)
                    rcd = dram.tile([128, LH // 128], F16, name="rcd",
                                    tag=f"rcd{t}_{e}")
                    nc.gpsimd.dma_start(out=rcd, in_=rsc)
                    rb = attn.tile([64, LH], F16, name="rb", tag=f"rb{e}",
                                   bufs=1)
                    nc.scalar.dma_start(
                        out=rb,
                        in_=bass.AP(tensor=rcd.tensor, offset=rcd.offset,
                                    ap=[[0, 64], [1, LH]]))
                    nc.vector.tensor_mul(
                        out=o_res[t][64 * e:64 * (e + 1), :],
                        in0=osb_raw[0:64, :], in1=rb)

            # interleaved schedule: the remaining projection psum-groups
            # are spread as fillers inside the attention jc loops (one
            # ~1.7us group per iteration) so the exp stream starts early
            # and the PE never drains while ACT stays saturated
            emit_qproj(0)
            emit_kproj(0)
            emit_vproj(0)

            def qg(mo, n):
                return lambda: emit_qproj_group(mo, n)

            def kg(mo, n):
                return lambda: emit_kproj_group(mo, n)

            def vg(n, ml):
                return lambda: emit_vproj_group(n, ml)

            def mo_units(mo):
                return ([qg(mo, n) for n in range(2)]
                        + [kg(mo, n) for n in range(4)])

            v1 = [vg(1, ml) for ml in range(L // 128)]
            # each unit must be emitted at least one pair before the pair
            # that consumes its output (program order = staleness order)
            fillers_by_pair = [
                mo_units(1) + v1[0:4],       # pair 0
                mo_units(2) + v1[4:8],       # pair 1
                mo_units(3) + v1[8:12],      # pair 2
                mo_units(4) + v1[12:16],     # pair 3
                mo_units(5),                 # pair 4
                mo_units(6),                 # pair 5
                mo_units(7),                 # pair 6
                [],
            ]
            for t in range(NPAIR):
                emit_pair(t, fillers_by_pair[t])

            # ---------------- output projection ----------------
            bo_sb = keep.tile([128, D], F32)
            nc.gpsimd.dma_start(
                out=bo_sb,
                in_=bass.AP(tensor=bo, offset=0, ap=[[0, 128], [1, D]]))
            wo_t = []
            for kc in range(NP):
                wt = oproj.tile([128, D], F16, name="wo_t", tag="wo", bufs=NP)
                nc.sync.dma_start(out=wt, in_=woT[128 * kc:128 * (kc + 1), :])
                wo_t.append(wt)
            for mi in range(LH // 128):
                for n in range(2):
                    y_ps = ps_tile("ps_s" if n % 2 == 0 else "ps_o")
                    for kc in range(NP):
                        nc.tensor.matmul(
                            y_ps[:, 0:512],
                            lhsT=o_res[kc][:, 128 * mi:128 * (mi + 1)],
                            rhs=wo_t[kc][:, 512 * n:512 * (n + 1)],
                            start=(kc == 0), stop=(kc == NP - 1))
                    ysb = oproj.tile([128, 512], F32, name="ysb", tag="ysb",
                                     bufs=4)
                    nc.vector.tensor_add(out=ysb, in0=y_ps[:, 0:512],
                                         in1=bo_sb[:, 512 * n:512 * (n + 1)])
                    nc.gpsimd.dma_start(
                        out=yT[128 * mi:128 * (mi + 1),
                               512 * n:512 * (n + 1)],
                        in_=ysb)

    nc.compile()
    return nc


_NC_CACHE = []


def kernel_with_results(x, wq, wk, wv, wo, bo, **run_kwargs):
    x = np.asarray(x, dtype=np.float32)
    wqT = np.ascontiguousarray(np.asarray(wq, dtype=np.float32).T,
                               dtype=np.float16)
    wkT = np.ascontiguousarray(np.asarray(wk, dtype=np.float32).T,
                               dtype=np.float16)
    wvT = np.ascontiguousarray(np.asarray(wv, dtype=np.float32).T,
                               dtype=np.float16)
    woT = np.ascontiguousarray(np.asarray(wo, dtype=np.float32).T,
                               dtype=np.float16)
    bo2 = np.asarray(bo, dtype=np.float32).reshape(1, D)

    if not _NC_CACHE:
        _NC_CACHE.append(build_nc())
    nc = _NC_CACHE[0]

    in_maps = []
    for c in range(NCORES):
        b, half = divmod(c, 2)
        xb = np.ascontiguousarray(x[b, :, 0, :]).astype(np.float16)
        in_maps.append({
            "x": xb,
            "xq": np.ascontiguousarray(xb[:, half * LH:(half + 1) * LH]),
            "wqT": wqT, "wkT": wkT, "wvT": wvT, "woT": woT, "bo": bo2,
            "ones16": np.ones((1, NJC), dtype=np.float16),
            "zeros16": np.zeros((1, L), dtype=np.float16),
        })

    kres = run_bass_kernel_spmd(nc, in_maps, list(range(NCORES)), **run_kwargs)

    out = np.empty((B, D, 1, L), dtype=np.float32)
    for c in range(NCORES):
        b, half = divmod(c, 2)
        out[b, :, 0, half * LH:(half + 1) * LH] = kres.results[c]["yT"].T
    return out, kres


def kernel(x, wq, wk, wv, wo, bo):
    out, _ = kernel_with_results(x, wq, wk, wv, wo, bo)
    return out

